# revision 31
# baseline (speedup 1.0000x reference)
"""MGDPR (gnn_message_passing) Trainium2 kernel, 8 NeuronCores.

Sharding: nodes row-sharded 4-way within each batch element; cores 0-3 own
batch 0, cores 4-7 own batch 1 (375 dest nodes each). Source nodes live in a
padded space (384 per shard = 3x128 tiles) so gather shards align with
128-partition tiles. All per-node tensors are channel-major on chip
([C, nodes]); the diffusion matmul contracts source nodes on partitions using
node-major h tiles produced by PE transposes of the (channel-major) gathered
h. h is re-gathered across the 4 cores of each batch after layers 0 and 1 via
a channel-major AllGather (no pre-transpose needed). No 8-core barrier: the
4-core AllGathers absorb launch skew within each group only. All matmuls are
bf16 (f32r matmuls run in slow fp32-HIGH mode on hw) except the GroupNorm
stats which stay f32r for variance accuracy. h_prime never depends on node
data (zeros init + per-channel affine), so it folds into a per-layer bias.
"""

import numpy as np

try:
    import concourse.bass as bass
except ImportError:
    import sys

    sys.path.insert(0, "/opt/trn_rl_repo")
    import concourse.bass as bass

import concourse.mybir as mybir
import concourse.tile as tile
from concourse import bacc
from concourse.bass_utils import run_bass_kernel_spmd

B, N, T, DIN, C, R, K, L, H, OUT = 2, 1500, 20, 32, 128, 5, 5, 3, 4, 2
HD = C // H
EPS = 1e-5
NCORES = 8
NS = N // 4          # 375 real nodes per shard
NSP = NS + 1         # dest cols per core (padded even)
NSH = 384            # padded source nodes per shard (3x128)
NT = 12              # source tiles (4*384/128)
MPAD = 4 * NSH
CW = NSP // 2        # chunk width (188)
RG = [[0, 1, 2, 3], [4, 5, 6, 7]]
F32R = mybir.dt.float32r
F32 = mybir.dt.float32
BF16 = mybir.dt.bfloat16
F8 = mybir.dt.float8e4
AF = mybir.ActivationFunctionType

_NC_CACHE = {}


def _build_nc():
    if "nc" in _NC_CACHE:
        return _NC_CACHE["nc"]
    nc = bacc.Bacc(None, target_bir_lowering=False, debug=False, num_devices=NCORES)

    # ---- per-core inputs ----
    adjt = nc.dram_tensor("adjt", [2, 128, (NT // 2) * R * NSP], F8, kind="ExternalInput")
    xt = nc.dram_tensor("xt", [DIN + 1, MPAD], BF16, kind="ExternalInput")
    # ---- replicated consts (host-prelaid in SBUF layout, partition-first) ----
    # consts grouped by dtype into single tensors (one DMA each):
    # cb (bf16): wp | qkvo | mh | w2at | ident | ow2t
    # cf (f32):  mmu | cols | cols2
    # c8 (f8):   ident8 | ow1t
    NB = L * R * C + L * 4 * C + C + L * C + C + OUT
    NF = (4 * L + 3) + 4 * L
    N8 = C + C
    cb_d = nc.dram_tensor("cb", [C, NB], BF16, kind="ExternalInput")
    cf_d = nc.dram_tensor("cf", [C, NF], F32, kind="ExternalInput")
    mmu_d = nc.dram_tensor("mmu", [C, C], F32R, kind="ExternalInput")
    c8_d = nc.dram_tensor("c8", [C, N8], F8, kind="ExternalInput")
    embt_d = nc.dram_tensor("embt", [DIN + 1, C], BF16, kind="ExternalInput")
    rows_d = nc.dram_tensor("rows", [1, 4 * L * C], BF16, kind="ExternalInput")

    outt = nc.dram_tensor("outt", [OUT, NSP], F32R, kind="ExternalOutput")

    # gather buffers (internal DRAM), channel-major, split per chunk:
    # A = shard nodes [0,256) (two aligned tiles), B = [256,384) (one tile)
    gA_in = [nc.dram_tensor(f"gA_in_{l}", [C, 256], F8) for l in range(2)]
    gA_out = [nc.dram_tensor(f"gA_out_{l}", [4 * C, 256], F8) for l in range(2)]
    gB_in = [nc.dram_tensor(f"gB_in_{l}", [C, 128], F8) for l in range(2)]
    gB_out = [nc.dram_tensor(f"gB_out_{l}", [4 * C, 128], F8) for l in range(2)]
    gw_in = nc.dram_tensor("gw_in", [C, 8], F8)
    gw_out = nc.dram_tensor("gw_out", [4 * C, 8], F8)

    with tile.TileContext(nc) as tc:
        with (
            tc.tile_pool(name="persist", bufs=1) as pers,
            tc.tile_pool(name="ret", bufs=2) as ret,
            tc.tile_pool(name="zwork", bufs=6) as zwork,
            tc.tile_pool(name="big", bufs=2) as big,
            tc.tile_pool(name="misc", bufs=2) as misc,
            tc.tile_pool(name="pz", bufs=3, space="PSUM") as pz,
            tc.tile_pool(name="pm", bufs=2, space="PSUM") as pm,
            tc.tile_pool(name="pp", bufs=3, space="PSUM") as pp,
        ):
            # ---------- resident tensors ----------
            adjsb = pers.tile([128, NT, R, NSP], F8, tag="adjsb")
            hnat = pers.tile([128, NT * 128], F8, tag="hnat")
            hshA = pers.tile([128, 4 * 256], F8, tag="hshA")
            hshB = pers.tile([128, 4 * 128], F8, tag="hshB")
            xtsb = pers.tile([DIN + 1, MPAD], BF16, tag="xtsb")
            embtsb = pers.tile([DIN + 1, C], BF16, tag="embtsb")
            cbsb = pers.tile([C, NB], BF16, tag="cbsb")
            cfsb = pers.tile([C, NF], F32, tag="cfsb")
            mmusb = pers.tile([C, C], F32R, tag="mmusb")
            c8sb = pers.tile([C, N8], F8, tag="c8sb")
            rowsb = pers.tile([1, 4 * L * C], BF16, tag="rowsb")
            onesb = pers.tile([1, NSP], BF16, tag="onesb")
            zerosb = pers.tile([128, NSH], F8, tag="zerosb")
            o = [0]

            def _nxt(w):
                a = o[0]; o[0] += w
                return a

            wpsb = cbsb[:, _nxt(L * R * C) : o[0]]
            qkvosb = cbsb[:, _nxt(L * 4 * C) : o[0]]
            mhsb = cbsb[:, _nxt(C) : o[0]]
            w2atsb = cbsb[:, _nxt(L * C) : o[0]]
            identsb = cbsb[:, _nxt(C) : o[0]]
            ow2tsb = cbsb[:, _nxt(OUT) : o[0]]
            o[0] = 0
            colsb = cfsb[:, _nxt(4 * L + 3) : o[0]]
            colsb2 = cfsb[:, _nxt(4 * L) : o[0]]
            o[0] = 0
            ident8sb = c8sb[:, _nxt(C) : o[0]]
            ow1tsb = c8sb[:, _nxt(C) : o[0]]

            # ---------- input DMA: adj halves first, grouped consts ----------
            adjflat = adjsb.rearrange("p mt r j -> p (mt r j)")
            HSZ = (NT // 2) * R * NSP
            nc.sync.dma_start(adjflat[:, 0:HSZ], adjt[0])
            nc.scalar.dma_start(xtsb[:], xt[:, :])
            nc.scalar.dma_start(embtsb[:], embt_d[:, :])
            nc.scalar.dma_start(adjflat[:, HSZ : 2 * HSZ], adjt[1])
            nc.gpsimd.dma_start(cfsb[:], cf_d[:, :])
            nc.gpsimd.dma_start(mmusb[:], mmu_d[:, :])
            nc.gpsimd.dma_start(cbsb[:], cb_d[:, :])
            nc.gpsimd.dma_start(c8sb[:], c8_d[:, :])
            nc.gpsimd.dma_start(rowsb[:], rows_d[:, :])

            nc.vector.memset(onesb[:], 1.0)
            nc.vector.memset(zerosb[:], 0.0)
            # tiny AllGather fired immediately: pays the first-collective
            # setup + absorbs launch skew while the adj DMA streams in
            nc.sync.dma_start(gw_in[:, :], zerosb[:, 0:8])
            nc.gpsimd.collective_compute(
                "AllGather", mybir.AluOpType.bypass,
                replica_groups=RG,
                ins=[gw_in[:, :].opt()],
                outs=[gw_out[:, :].opt()],
            )
            # pre-zero the B-chunk gather inputs so pad cols are exact
            # zeros, not junk DRAM that could be NaN
            nc.gpsimd.dma_start(gB_in[0][:, :], zerosb[:, 0:128])
            nc.gpsimd.dma_start(gB_in[1][:, :], zerosb[:, 0:128])
            # dummy matmul buzz: ramps the PE clock gate to 2.4 GHz during
            # the input-DMA window so L0 diffusion starts at full speed
            for i in range(14):
                jw = pz.tile([128, NSH], F32, name=f"warm{i}", tag="zs")
                nc.tensor.matmul(jw[:, 0:NSH], zerosb[:, 0:128], zerosb[:],
                                 start=True, stop=True, skip_group_check=True)

            def col(i):
                return colsb[:, i : i + 1]

            def row(l, j):
                return rowsb[0:1, (4 * l + j) * C : (4 * l + j + 1) * C]

            wp3 = wpsb.rearrange("p (l r co) -> p l r co", l=L, r=R)
            qk4 = qkvosb.rearrange("p (l i co) -> p l i co", l=L, i=4)
            w2a3 = w2atsb.rearrange("p (l co) -> p l co", l=L)

            def blip(src):
                """Tiny junk matmul chained off a chain tile: keeps the PE's
                HAM activity window busy through DVE/ACT-only stretches so the
                clock gate stays at 2.4 GHz. Output is never read."""
                jp = pz.tile([128, 8], F32, name="blip", tag="zs")
                nc.tensor.matmul(jp[:], identsb[:], src[:, 0:8],
                                 start=True, stop=True, skip_group_check=True)

            copy_eng = [0]

            def copy_alt(dst, src):
                if copy_eng[0] % 2 == 0:
                    nc.vector.tensor_copy(dst, src)
                else:
                    nc.scalar.copy(dst, src)
                copy_eng[0] += 1

            # ---------- h0 = embedding (node-major, source-node space) ----------
            for mt in range(NT):
                ep = pp.tile([128, 128], F32, tag="ps")
                nc.tensor.matmul(
                    ep[:], xtsb[:, mt * 128 : (mt + 1) * 128], embtsb[:],
                    start=True, stop=True,
                )
                copy_alt(hnat[:, mt * 128 : (mt + 1) * 128], ep[:])

            # ---------- layer machinery ----------
            CHUNKS = [(0, 256), (256, NSP - 256)]
            RGROUPS = [(0, 1, 2), (3, 4)]
            EARLY = [3 * s + t for s in range(4) for t in (0, 1)]
            LATE = [3 * s + 2 for s in range(4)]

            def diff_mms(rg, c0, cw, mts=None, zps=None, first=True, last=True):
                """Emit adjacency matmuls for one r-group over the given
                source tiles; the accumulation group can be split across
                calls via first/last."""
                if mts is None:
                    mts = range(NT)
                if zps is None:
                    zps = {r: pz.tile([128, cw], F32, name=f"zps{r}", tag="zs")
                           for r in rg}
                mts = list(mts)
                for i, mt in enumerate(mts):
                    for r in rg:
                        nc.tensor.matmul(
                            zps[r][:],
                            hnat[:, mt * 128 : (mt + 1) * 128],
                            adjsb[:, mt, r, c0 : c0 + cw],
                            start=(first and i == 0),
                            stop=(last and i == len(mts) - 1),
                            skip_group_check=True,
                        )
                return zps

            def diff_proj(l, rg, zps, mps, cw):
                for r in rg:
                    zsb = zwork.tile([128, cw], BF16, tag="zsb")
                    copy_alt(zsb[:], zps[r][:])
                    nc.tensor.matmul(
                        mps[:], wp3[:, l, r, :], zsb[:],
                        start=(r == 0), stop=(r == R - 1),
                        skip_group_check=True,
                    )

            def diffusion(l, c0, cw):
                """Returns mps PSUM tile [128, cw] with merged diffusion."""
                mps = pm.tile([128, cw], F32, tag="mps")
                for rg in RGROUPS:
                    zps = diff_mms(rg, c0, cw)
                    diff_proj(l, rg, zps, mps, cw)
                return mps

            def ret_head(l, mps, c0, cw):
                """relu + q/k/v projections. Returns (qps, ksb, vsb)."""
                hdT = ret.tile([128, cw], BF16, tag="hdT")
                nc.scalar.activation(
                    hdT[:], mps[:], AF.Relu, bias=col(4 * l + 0), scale=1.0
                )
                qps = pp.tile([128, cw], F32, tag="ps")
                nc.tensor.matmul(qps[:], qk4[:, l, 0, :], hdT[:],
                                 start=True, stop=False, skip_group_check=True)
                nc.tensor.matmul(qps[:], row(l, 0), onesb[0:1, c0 : c0 + cw],
                                 start=False, stop=True, skip_group_check=True)
                kps = pp.tile([128, cw], F32, tag="ps")
                nc.tensor.matmul(kps[:], qk4[:, l, 1, :], hdT[:],
                                 start=True, stop=True)
                ksb = ret.tile([128, cw], F32R, tag="ksb")
                nc.scalar.activation(ksb[:], kps[:], AF.Identity, bias=row_as_col(l, 1))
                vps = pp.tile([128, cw], F32, tag="ps")
                nc.tensor.matmul(vps[:], qk4[:, l, 2, :], hdT[:],
                                 start=True, stop=True)
                vsb = ret.tile([128, cw], F32R, tag="vsb")
                nc.vector.tensor_scalar_add(vsb[:], vps[:], row_as_col(l, 2))
                return qps, ksb, vsb

            def row_as_col(l, j):
                # kb/vb applied as per-partition activation-bias columns
                return colsb2[:, (4 * l + j) : (4 * l + j) + 1]

            def ret_tail(l, qps, ksb, vsb, c0, cw, dest):
                """Retention tail as 4 emission segments so the caller can
                interleave them with another chunk's diffusion matmuls (engine
                streams execute in emission order)."""
                st = {}

                def seg1():
                    st["qk"] = ret.tile([128, cw], BF16, name="qk", tag="qk")
                    nc.vector.tensor_mul(st["qk"][:], ksb[:], qps[:])
                    st["sbps"] = pp.tile([128, cw], F32, name="sbps", tag="ps")
                    nc.tensor.matmul(st["sbps"][:], mhsb[:], st["qk"][:],
                                     start=True, stop=True)
                    st["osb"] = ret.tile([128, cw], BF16, name="osb", tag="osb")
                    nc.vector.tensor_mul(st["osb"][:], vsb[:], st["sbps"][:])
                    blip(st["qk"])

                def seg2():
                    st["o2ps"] = pp.tile([128, cw], F32, name="o2ps", tag="ps")
                    nc.tensor.matmul(st["o2ps"][:], qk4[:, l, 3, :], st["osb"][:],
                                     start=True, stop=False, skip_group_check=True)
                    nc.tensor.matmul(st["o2ps"][:], row(l, 3),
                                     onesb[0:1, c0 : c0 + cw],
                                     start=False, stop=True, skip_group_check=True)
                    st["sq"] = ret.tile([128, cw], F32R, name="sq", tag="sq")
                    nc.scalar.activation(st["sq"][:], st["o2ps"][:], AF.Square)
                    st["o2sb"] = ret.tile([128, cw], F32R, name="o2sb", tag="o2sb")
                    nc.vector.tensor_copy(st["o2sb"][:], st["o2ps"][:])

                def seg3():
                    mups = pp.tile([128, cw], F32, tag="ps")
                    nc.tensor.matmul(mups[:], mmusb[:], st["o2sb"][:],
                                     start=True, stop=True)
                    msps = pp.tile([128, cw], F32, tag="ps")
                    nc.tensor.matmul(msps[:], mmusb[:], st["sq"][:],
                                     start=True, stop=True)
                    mu2 = ret.tile([128, cw], F32R, tag="mu2")
                    nc.scalar.activation(mu2[:], mups[:], AF.Square)
                    tsb = ret.tile([128, cw], BF16, tag="tsb")
                    nc.vector.tensor_sub(tsb[:], st["o2sb"][:], mups[:])
                    varsb = ret.tile([128, cw], F32R, tag="varsb")
                    nc.vector.tensor_sub(varsb[:], msps[:], mu2[:])
                    rstd = ret.tile([128, cw], BF16, tag="rstd")
                    # 1/sqrt(var+eps) in one table-resident activation; the
                    # abs is a no-op since var+eps > 0
                    nc.scalar.activation(rstd[:], varsb[:],
                                         AF.Abs_reciprocal_sqrt, bias=col(4 * L))
                    blip(tsb)
                    # hr = (o2-mu)*rstd*gn_g + gn_b; the gn_b term is folded
                    # into the w2 bias on the host, so one stt does the rest
                    st["hrT"] = ret.tile([128, cw], BF16, name="hrT", tag="hrT")
                    nc.vector.scalar_tensor_tensor(
                        st["hrT"][:], tsb[:], col(4 * l + 2), rstd[:],
                        mybir.AluOpType.mult, mybir.AluOpType.mult,
                    )

                def seg4():
                    h2ps = pp.tile([128, cw], F32, tag="ps")
                    nc.tensor.matmul(h2ps[:], w2a3[:, l, :], st["hrT"][:],
                                     start=True, stop=True)
                    nc.scalar.activation(
                        dest, h2ps[:], AF.Relu,
                        bias=col(4 * l + 1), scale=1.0,
                    )

                return seg1, seg2, seg3, seg4

            def transpose_tiles(srcsb, pairs):
                """PE-transpose [c, node] 128-blocks into node-major hnat
                tiles. pairs = [(src_block_idx, hnat_tile_idx), ...]."""
                for sb_i, mt in pairs:
                    # fp8 PE transpose requires output element step 2
                    tp = pp.tile([128, 256], F8, tag="ps")
                    nc.tensor.transpose(
                        tp[:, 0:256:2], srcsb[:, sb_i * 128 : (sb_i + 1) * 128],
                        ident8sb[:],
                    )
                    copy_alt(hnat[:, mt * 128 : (mt + 1) * 128], tp[:, 0:256:2])

            # ---------- layers ----------
            for l in range(L):
                (a0, aw), (b0, bw) = CHUNKS
                if l > 0:
                    # the A-half of the gather (2 of 3 tiles per shard) lands
                    # first: rebuild+diffuse those while AG-B is in flight
                    hshA3 = hshA.rearrange("p (s j) -> p s j", s=4)
                    hshB3 = hshB.rearrange("p (s j) -> p s j", s=4)
                    nc.sync.dma_start(
                        hshA3[:, 0:2, :],
                        gA_out[l - 1][0:256, :].rearrange("(s c) j -> c s j", s=2),
                    )
                    nc.scalar.dma_start(
                        hshA3[:, 2:4, :],
                        gA_out[l - 1][256:512, :].rearrange("(s c) j -> c s j", s=2),
                    )
                    nc.gpsimd.dma_start(
                        hshB3[:, 0:2, :],
                        gB_out[l - 1][0:256, :].rearrange("(s c) j -> c s j", s=2),
                    )
                    nc.sync.dma_start(
                        hshB3[:, 2:4, :],
                        gB_out[l - 1][256:512, :].rearrange("(s c) j -> c s j", s=2),
                    )
                    transpose_tiles(
                        hshA, [(2 * s + t, 3 * s + t) for s in range(4)
                               for t in (0, 1)]
                    )

                if l == 2:
                    hnT_full = big.tile([C, NSP], F8, tag="hnT")
                    destA = hnT_full[:, a0 : a0 + aw]
                    destB = hnT_full[:, b0 : b0 + bw]
                else:
                    hnTa = big.tile([C, 256], F8, tag="hnTa")
                    hnTb = big.tile([C, 128], F8, tag="hnTb")
                    destA = hnTa[:, :]
                    destB = hnTb[:, 0:bw]

                mpsA = pm.tile([128, aw], F32, tag="mps")
                if l == 0:
                    zA1 = diff_mms(RGROUPS[0], a0, aw)
                else:
                    # start on the early tiles; transpose the B-half tiles as
                    # soon as AG-B delivers, then finish the accumulation
                    zA1 = diff_mms(RGROUPS[0], a0, aw, mts=EARLY, last=False)
                    transpose_tiles(hshB, [(s, 3 * s + 2) for s in range(4)])
                    diff_mms(RGROUPS[0], a0, aw, mts=LATE, zps=zA1, first=False)
                diff_proj(l, RGROUPS[0], zA1, mpsA, aw)
                zA2 = diff_mms(RGROUPS[1], a0, aw)
                diff_proj(l, RGROUPS[1], zA2, mpsA, aw)
                qA = ret_head(l, mpsA, a0, aw)
                # chunk B diffusion interleaved with chunk A retention tail:
                # A's DVE/ACT chain runs while the PE grinds B's adjacency
                # matmuls; A's few PE hops slot between B's r-groups.
                s1, s2, s3, s4 = ret_tail(l, *qA, a0, aw, destA)
                mpsB = pm.tile([128, bw], F32, tag="mps")
                zB1 = diff_mms(RGROUPS[0], b0, bw)
                s1()
                diff_proj(l, RGROUPS[0], zB1, mpsB, bw)
                s2()
                zB2 = diff_mms(RGROUPS[1], b0, bw)
                s3()
                diff_proj(l, RGROUPS[1], zB2, mpsB, bw)
                s4()
                if l < 2:
                    # ship the A half while chunk B's retention chain runs
                    nc.sync.dma_start(gA_in[l][:, :], hnTa[:])
                    nc.gpsimd.collective_compute(
                        "AllGather", mybir.AluOpType.bypass,
                        replica_groups=RG,
                        ins=[gA_in[l][:, :].opt()],
                        outs=[gA_out[l][:, :].opt()],
                    )
                qB = ret_head(l, mpsB, b0, bw)
                t1, t2, t3, t4 = ret_tail(l, *qB, b0, bw, destB)
                t1(); t2(); t3(); t4()

                if l < 2:
                    nc.scalar.dma_start(gB_in[l][:, 0:bw], hnTb[:, 0:bw])
                    nc.gpsimd.collective_compute(
                        "AllGather", mybir.AluOpType.bypass,
                        replica_groups=RG,
                        ins=[gB_in[l][:, :].opt()],
                        outs=[gB_out[l][:, :].opt()],
                    )
                else:
                    # final head
                    hmps = pp.tile([128, NSP], F32, tag="ps")
                    nc.tensor.matmul(hmps[:], ow1tsb[:], hnT_full[:],
                                     start=True, stop=True)
                    hmsb = misc.tile([C, NSP], BF16, tag="hmsb")
                    nc.scalar.activation(
                        hmsb[:], hmps[:], AF.Relu, bias=col(4 * L + 1)
                    )
                    oops = pp.tile([OUT, NSP], F32, tag="ps")
                    nc.tensor.matmul(oops[:], ow2tsb[:], hmsb[:],
                                     start=True, stop=True)
                    oosb = misc.tile([OUT, NSP], F32R, tag="oosb")
                    nc.scalar.activation(
                        oosb[:], oops[:], AF.Identity,
                        bias=colsb[0:OUT, 4 * L + 2 : 4 * L + 3],
                    )
                    nc.sync.dma_start(outt[:, :], oosb[:])

    nc.finalize()
    _NC_CACHE["nc"] = nc
    return nc


def _prep(inputs):
    import ml_dtypes

    bf16 = ml_dtypes.bfloat16
    f8 = ml_dtypes.float8_e4m3
    f32 = np.float32

    def g(name):
        return np.asarray(inputs[name], f32)

    x, adj = g("x"), g("adj_list")
    alpha, transition = g("alpha"), g("transition")
    conv_w, conv_b = g("conv_w"), g("conv_b")
    w1, b1, eb1 = g("w1"), g("b1"), g("eb1")
    w2, b2, eb2 = g("w2"), g("b2"), g("eb2")

    a = alpha - alpha.max(-1, keepdims=True)
    e = np.exp(a)
    srow = (e / e.sum(-1, keepdims=True)).sum(-1)          # [L,R]
    Wm = transition.mean(axis=2)                            # [L,R,C,C]
    Wp = (conv_w * srow)[:, :, None, None] * np.swapaxes(Wm, -1, -2)

    hp = np.zeros((C,), f32)
    b2eff = np.zeros((L, C), f32)
    for l in range(L):
        # gn_b's contribution through w2 is folded in here so the kernel's
        # GN affine is a single (x-mu)*rstd*gn_g op
        b2eff[l] = b2[l] + eb2[l] + w2[l][:, C:] @ hp + w2[l][:, :C] @ g("gn_b")[l]
        hp = np.maximum(hp @ w1[l].T + b1[l] + eb1[l], 0.0).astype(f32)

    qkvo = np.stack(
        [np.swapaxes(g(w), -1, -2) for w in ("qw", "kw", "vw", "ow")], axis=1
    )  # [L,4,C,C] in lhsT layout

    hid = np.arange(C) // HD
    same = (hid[:, None] == hid[None, :]).astype(f32)       # [C,C]

    cols = np.zeros((C, 4 * L + 3), f32)
    cols2 = np.zeros((C, 4 * L), f32)
    rows = np.zeros((1, 4 * L * C), f32)
    for l in range(L):
        cols[:, 4 * l + 0] = conv_b[l]
        cols[:, 4 * l + 1] = b2eff[l]
        cols[:, 4 * l + 2] = g("gn_g")[l]
        cols[:, 4 * l + 3] = g("gn_b")[l]
        for j, nm in enumerate(("qb", "kb", "vb", "ob")):
            cols2[:, 4 * l + j] = g(nm)[l]
            rows[0, (4 * l + j) * C : (4 * l + j + 1) * C] = g(nm)[l]
    cols[:, 4 * L] = EPS
    cols[:, 4 * L + 1] = g("out_b1")
    cols[:OUT, 4 * L + 2] = g("out_b2")

    wp_h = np.ascontiguousarray(
        (Wp / 16.0).transpose(2, 0, 1, 3).reshape(C, L * R * C)
    )
    qkvo_h = np.ascontiguousarray(qkvo.transpose(2, 0, 1, 3).reshape(C, L * 4 * C))
    w2at_h = np.ascontiguousarray(
        np.swapaxes(w2[:, :, :C], -1, -2).transpose(1, 0, 2).reshape(C, L * C)
    )
    eye = np.eye(C, dtype=f32)
    consts = {
        "cb": np.concatenate(
            [wp_h, qkvo_h, same, w2at_h, eye, g("out_w2").T], axis=1
        ).astype(bf16),
        "cf": np.concatenate([cols, cols2], axis=1).astype(f32),
        "mmu": (same / HD).astype(f32),
        "c8": np.concatenate([eye, g("out_w1").T], axis=1).astype(f8),
        "embt": np.concatenate(
            [g("emb_w").T, g("emb_b")[None, :]], axis=0
        ).astype(bf16),
        "rows": rows.astype(bf16),
    }

    xlast = x[:, :, -1, :]                                   # [B,N,DIN]
    in_maps = []
    for k in range(NCORES):
        b, s = k // 4, k % 4
        asub = adj[b][:, s * NS : (s + 1) * NS, :]           # [R,NS,N] dest rows
        ap = np.zeros((R, NSP, MPAD), f32)
        for s2 in range(4):
            ap[:, :NS, s2 * NSH : s2 * NSH + NS] = asub[:, :, s2 * NS : (s2 + 1) * NS]
        a3 = ap.transpose(2, 0, 1).reshape(NT, 128, R, NSP)  # [mt, mi, R, NSP]
        a3 = (
            a3.reshape(2, NT // 2, 128, R * NSP)
            .transpose(0, 2, 1, 3)                           # [half, mi, 6, R*NSP]
            .reshape(2, 128, (NT // 2) * R * NSP)
        )
        xt = np.zeros((DIN + 1, MPAD), f32)
        for s2 in range(4):
            xt[:DIN, s2 * NSH : s2 * NSH + NS] = xlast[b, s2 * NS : (s2 + 1) * NS].T
        xt[DIN, :] = 1.0
        in_maps.append(
            dict(consts, adjt=(np.ascontiguousarray(a3) * 16.0).astype(f8),
                 xt=xt.astype(bf16))
        )
    return in_maps


def kernel(**inputs):
    nc = _build_nc()
    in_maps = _prep(inputs)
    res = run_bass_kernel_spmd(nc, in_maps, core_ids=list(range(NCORES)))
    out = np.zeros((B, N, OUT), np.float32)
    for k in range(NCORES):
        b, s = k // 4, k % 4
        out[b, s * NS : (s + 1) * NS, :] = res.results[k]["outt"][:, :NS].T
    return out


# revision 32
# speedup vs baseline: 1.1504x; 1.1504x over previous
"""MGDPR (gnn_message_passing) Trainium2 kernel, 8 NeuronCores.

Sharding: nodes row-sharded 4-way within each batch element; cores 0-3 own
batch 0, cores 4-7 own batch 1 (375 dest nodes each). Source nodes live in a
padded space (384 per shard = 3x128 tiles) so gather shards align with
128-partition tiles. All per-node tensors are channel-major on chip
([C, nodes]); the diffusion matmul contracts source nodes on partitions using
node-major h tiles produced by PE transposes of the (channel-major) gathered
h. h is re-gathered across the 4 cores of each batch after layers 0 and 1 via
a channel-major AllGather (no pre-transpose needed). No 8-core barrier: the
4-core AllGathers absorb launch skew within each group only. All matmuls are
bf16 (f32r matmuls run in slow fp32-HIGH mode on hw) except the GroupNorm
stats which stay f32r for variance accuracy. h_prime never depends on node
data (zeros init + per-channel affine), so it folds into a per-layer bias.
"""

import numpy as np

try:
    import concourse.bass as bass
except ImportError:
    import sys

    sys.path.insert(0, "/opt/trn_rl_repo")
    import concourse.bass as bass

import concourse.mybir as mybir
import concourse.tile as tile
from concourse import bacc
from concourse.bass_utils import run_bass_kernel_spmd

B, N, T, DIN, C, R, K, L, H, OUT = 2, 1500, 20, 32, 128, 5, 5, 3, 4, 2
HD = C // H
EPS = 1e-5
NCORES = 8
NS = N // 4          # 375 real nodes per shard
NSP = NS + 1         # dest cols per core (padded even)
NSH = 384            # padded source nodes per shard (3x128)
NT = 12              # source tiles (4*384/128)
MPAD = 4 * NSH
CW = NSP // 2        # chunk width (188)
RG = [[0, 1, 2, 3], [4, 5, 6, 7]]
F32R = mybir.dt.float32r
F32 = mybir.dt.float32
BF16 = mybir.dt.bfloat16
F8 = mybir.dt.float8e4
AF = mybir.ActivationFunctionType

_NC_CACHE = {}


def _build_nc():
    if "nc" in _NC_CACHE:
        return _NC_CACHE["nc"]
    nc = bacc.Bacc(None, target_bir_lowering=False, debug=False, num_devices=NCORES)

    # ---- per-core inputs ----
    adjt = nc.dram_tensor("adjt", [2, 128, (NT // 2) * R * NSP], F8, kind="ExternalInput")
    xt = nc.dram_tensor("xt", [DIN + 1, MPAD], BF16, kind="ExternalInput")
    # ---- replicated consts (host-prelaid in SBUF layout, partition-first) ----
    # consts grouped by dtype into single tensors (one DMA each):
    # cb (bf16): wp | qkvo | mh | w2at | ident | ow2t
    # cf (f32):  mmu | cols | cols2
    # c8 (f8):   ident8 | ow1t
    NB = L * R * C + L * 4 * C + C + L * C + C + OUT
    NF = (4 * L + 3) + 4 * L
    N8 = C + C
    cb_d = nc.dram_tensor("cb", [C, NB], BF16, kind="ExternalInput")
    cf_d = nc.dram_tensor("cf", [C, NF], F32, kind="ExternalInput")
    mmu_d = nc.dram_tensor("mmu", [C, C], F32R, kind="ExternalInput")
    c8_d = nc.dram_tensor("c8", [C, N8], F8, kind="ExternalInput")
    embt_d = nc.dram_tensor("embt", [DIN + 1, C], BF16, kind="ExternalInput")
    rows_d = nc.dram_tensor("rows", [1, 4 * L * C], BF16, kind="ExternalInput")

    outt = nc.dram_tensor("outt", [OUT, NSP], F32R, kind="ExternalOutput")

    # gather buffers (internal DRAM), channel-major, split per chunk:
    # A = shard nodes [0,256) (two aligned tiles), B = [256,384) (one tile)
    gA_in = [nc.dram_tensor(f"gA_in_{l}", [C, 256], F8) for l in range(2)]
    gA_out = [nc.dram_tensor(f"gA_out_{l}", [4 * C, 256], F8) for l in range(2)]
    gB_in = [nc.dram_tensor(f"gB_in_{l}", [C, 128], F8) for l in range(2)]
    gB_out = [nc.dram_tensor(f"gB_out_{l}", [4 * C, 128], F8) for l in range(2)]
    gw_in = nc.dram_tensor("gw_in", [C, 8], F8)
    gw_out = nc.dram_tensor("gw_out", [4 * C, 8], F8)

    with tile.TileContext(nc) as tc:
        with (
            tc.tile_pool(name="persist", bufs=1) as pers,
            tc.tile_pool(name="ret", bufs=2) as ret,
            tc.tile_pool(name="zwork", bufs=6) as zwork,
            tc.tile_pool(name="big", bufs=2) as big,
            tc.tile_pool(name="misc", bufs=2) as misc,
            tc.tile_pool(name="pz", bufs=3, space="PSUM") as pz,
            tc.tile_pool(name="pm", bufs=2, space="PSUM") as pm,
            tc.tile_pool(name="pp", bufs=3, space="PSUM") as pp,
        ):
            # ---------- resident tensors ----------
            adjsb = pers.tile([128, NT, R, NSP], F8, tag="adjsb")
            hnat = pers.tile([128, NT * 128], F8, tag="hnat")
            hshA = pers.tile([128, 4 * 256], F8, tag="hshA")
            hshB = pers.tile([128, 4 * 128], F8, tag="hshB")
            xtsb = pers.tile([DIN + 1, MPAD], BF16, tag="xtsb")
            embtsb = pers.tile([DIN + 1, C], BF16, tag="embtsb")
            cbsb = pers.tile([C, NB], BF16, tag="cbsb")
            cfsb = pers.tile([C, NF], F32, tag="cfsb")
            mmusb = pers.tile([C, C], F32R, tag="mmusb")
            c8sb = pers.tile([C, N8], F8, tag="c8sb")
            rowsb = pers.tile([1, 4 * L * C], BF16, tag="rowsb")
            onesb = pers.tile([1, NSP], BF16, tag="onesb")
            zerosb = pers.tile([128, NSH], F8, tag="zerosb")
            o = [0]

            def _nxt(w):
                a = o[0]; o[0] += w
                return a

            wpsb = cbsb[:, _nxt(L * R * C) : o[0]]
            qkvosb = cbsb[:, _nxt(L * 4 * C) : o[0]]
            mhsb = cbsb[:, _nxt(C) : o[0]]
            w2atsb = cbsb[:, _nxt(L * C) : o[0]]
            identsb = cbsb[:, _nxt(C) : o[0]]
            ow2tsb = cbsb[:, _nxt(OUT) : o[0]]
            o[0] = 0
            colsb = cfsb[:, _nxt(4 * L + 3) : o[0]]
            colsb2 = cfsb[:, _nxt(4 * L) : o[0]]
            o[0] = 0
            ident8sb = c8sb[:, _nxt(C) : o[0]]
            ow1tsb = c8sb[:, _nxt(C) : o[0]]

            # ---------- input DMA: adj halves first, grouped consts ----------
            adjflat = adjsb.rearrange("p mt r j -> p (mt r j)")
            HSZ = (NT // 2) * R * NSP
            nc.sync.dma_start(adjflat[:, 0:HSZ], adjt[0])
            nc.scalar.dma_start(xtsb[:], xt[:, :])
            nc.scalar.dma_start(embtsb[:], embt_d[:, :])
            nc.scalar.dma_start(adjflat[:, HSZ : 2 * HSZ], adjt[1])
            nc.gpsimd.dma_start(cfsb[:], cf_d[:, :])
            nc.gpsimd.dma_start(mmusb[:], mmu_d[:, :])
            nc.gpsimd.dma_start(cbsb[:], cb_d[:, :])
            nc.gpsimd.dma_start(c8sb[:], c8_d[:, :])
            nc.gpsimd.dma_start(rowsb[:], rows_d[:, :])

            nc.vector.memset(onesb[:], 1.0)
            nc.vector.memset(zerosb[:], 0.0)
            # pre-zero the B-chunk gather inputs so pad cols are exact
            # zeros, not junk DRAM that could be NaN
            nc.gpsimd.dma_start(gB_in[0][:, :], zerosb[:, 0:128])
            nc.gpsimd.dma_start(gB_in[1][:, :], zerosb[:, 0:128])
            # dummy matmul buzz: ramps the PE clock gate to 2.4 GHz during
            # the input-DMA window so L0 diffusion starts at full speed
            for i in range(14):
                jw = pz.tile([128, NSH], F32, name=f"warm{i}", tag="zs")
                nc.tensor.matmul(jw[:, 0:NSH], zerosb[:, 0:128], zerosb[:],
                                 start=True, stop=True, skip_group_check=True)

            def col(i):
                return colsb[:, i : i + 1]

            def row(l, j):
                return rowsb[0:1, (4 * l + j) * C : (4 * l + j + 1) * C]

            wp3 = wpsb.rearrange("p (l r co) -> p l r co", l=L, r=R)
            qk4 = qkvosb.rearrange("p (l i co) -> p l i co", l=L, i=4)
            w2a3 = w2atsb.rearrange("p (l co) -> p l co", l=L)

            def blip(src):
                """Tiny junk matmul chained off a chain tile: keeps the PE's
                HAM activity window busy through DVE/ACT-only stretches so the
                clock gate stays at 2.4 GHz. Output is never read."""
                jp = pz.tile([128, 8], F32, name="blip", tag="zs")
                nc.tensor.matmul(jp[:], identsb[:], src[:, 0:8],
                                 start=True, stop=True, skip_group_check=True)

            copy_eng = [0]

            def copy_alt(dst, src):
                if copy_eng[0] % 2 == 0:
                    nc.vector.tensor_copy(dst, src)
                else:
                    nc.scalar.copy(dst, src)
                copy_eng[0] += 1

            # ---------- h0 = embedding (node-major, source-node space) ----------
            for mt in range(NT):
                ep = pp.tile([128, 128], F32, tag="ps")
                nc.tensor.matmul(
                    ep[:], xtsb[:, mt * 128 : (mt + 1) * 128], embtsb[:],
                    start=True, stop=True,
                )
                copy_alt(hnat[:, mt * 128 : (mt + 1) * 128], ep[:])

            # ---------- layer machinery ----------
            CHUNKS = [(0, 256), (256, NSP - 256)]
            RGROUPS = [(0, 1, 2), (3, 4)]
            EARLY = [3 * s + t for s in range(4) for t in (0, 1)]
            LATE = [3 * s + 2 for s in range(4)]

            def diff_mms(rg, c0, cw, mts=None, zps=None, first=True, last=True):
                """Emit adjacency matmuls for one r-group over the given
                source tiles; the accumulation group can be split across
                calls via first/last."""
                if mts is None:
                    mts = range(NT)
                if zps is None:
                    zps = {r: pz.tile([128, cw], F32, name=f"zps{r}", tag="zs")
                           for r in rg}
                mts = list(mts)
                for i, mt in enumerate(mts):
                    for r in rg:
                        nc.tensor.matmul(
                            zps[r][:],
                            hnat[:, mt * 128 : (mt + 1) * 128],
                            adjsb[:, mt, r, c0 : c0 + cw],
                            start=(first and i == 0),
                            stop=(last and i == len(mts) - 1),
                            skip_group_check=True,
                        )
                return zps

            def diff_proj(l, rg, zps, mps, cw):
                for r in rg:
                    zsb = zwork.tile([128, cw], BF16, tag="zsb")
                    copy_alt(zsb[:], zps[r][:])
                    nc.tensor.matmul(
                        mps[:], wp3[:, l, r, :], zsb[:],
                        start=(r == 0), stop=(r == R - 1),
                        skip_group_check=True,
                    )

            def diffusion(l, c0, cw):
                """Returns mps PSUM tile [128, cw] with merged diffusion."""
                mps = pm.tile([128, cw], F32, tag="mps")
                for rg in RGROUPS:
                    zps = diff_mms(rg, c0, cw)
                    diff_proj(l, rg, zps, mps, cw)
                return mps

            def ret_head(l, mps, c0, cw):
                """relu + q/k/v projections. Returns (qps, ksb, vsb)."""
                hdT = ret.tile([128, cw], BF16, tag="hdT")
                nc.scalar.activation(
                    hdT[:], mps[:], AF.Relu, bias=col(4 * l + 0), scale=1.0
                )
                qps = pp.tile([128, cw], F32, tag="ps")
                nc.tensor.matmul(qps[:], qk4[:, l, 0, :], hdT[:],
                                 start=True, stop=False, skip_group_check=True)
                nc.tensor.matmul(qps[:], row(l, 0), onesb[0:1, c0 : c0 + cw],
                                 start=False, stop=True, skip_group_check=True)
                kps = pp.tile([128, cw], F32, tag="ps")
                nc.tensor.matmul(kps[:], qk4[:, l, 1, :], hdT[:],
                                 start=True, stop=True)
                ksb = ret.tile([128, cw], F32R, tag="ksb")
                nc.scalar.activation(ksb[:], kps[:], AF.Identity, bias=row_as_col(l, 1))
                vps = pp.tile([128, cw], F32, tag="ps")
                nc.tensor.matmul(vps[:], qk4[:, l, 2, :], hdT[:],
                                 start=True, stop=True)
                vsb = ret.tile([128, cw], F32R, tag="vsb")
                nc.vector.tensor_scalar_add(vsb[:], vps[:], row_as_col(l, 2))
                return qps, ksb, vsb

            def row_as_col(l, j):
                # kb/vb applied as per-partition activation-bias columns
                return colsb2[:, (4 * l + j) : (4 * l + j) + 1]

            def ret_tail(l, qps, ksb, vsb, c0, cw, dest):
                """Retention tail as 4 emission segments so the caller can
                interleave them with another chunk's diffusion matmuls (engine
                streams execute in emission order)."""
                st = {}

                def seg1():
                    st["qk"] = ret.tile([128, cw], BF16, name="qk", tag="qk")
                    nc.vector.tensor_mul(st["qk"][:], ksb[:], qps[:])
                    st["sbps"] = pp.tile([128, cw], F32, name="sbps", tag="ps")
                    nc.tensor.matmul(st["sbps"][:], mhsb[:], st["qk"][:],
                                     start=True, stop=True)
                    st["osb"] = ret.tile([128, cw], BF16, name="osb", tag="osb")
                    nc.vector.tensor_mul(st["osb"][:], vsb[:], st["sbps"][:])
                    blip(st["qk"])

                def seg2():
                    st["o2ps"] = pp.tile([128, cw], F32, name="o2ps", tag="ps")
                    nc.tensor.matmul(st["o2ps"][:], qk4[:, l, 3, :], st["osb"][:],
                                     start=True, stop=False, skip_group_check=True)
                    nc.tensor.matmul(st["o2ps"][:], row(l, 3),
                                     onesb[0:1, c0 : c0 + cw],
                                     start=False, stop=True, skip_group_check=True)
                    st["sq"] = ret.tile([128, cw], F32R, name="sq", tag="sq")
                    nc.scalar.activation(st["sq"][:], st["o2ps"][:], AF.Square)
                    st["o2sb"] = ret.tile([128, cw], F32R, name="o2sb", tag="o2sb")
                    nc.vector.tensor_copy(st["o2sb"][:], st["o2ps"][:])

                def seg3():
                    mups = pp.tile([128, cw], F32, tag="ps")
                    nc.tensor.matmul(mups[:], mmusb[:], st["o2sb"][:],
                                     start=True, stop=True)
                    msps = pp.tile([128, cw], F32, tag="ps")
                    nc.tensor.matmul(msps[:], mmusb[:], st["sq"][:],
                                     start=True, stop=True)
                    mu2 = ret.tile([128, cw], F32R, tag="mu2")
                    nc.scalar.activation(mu2[:], mups[:], AF.Square)
                    tsb = ret.tile([128, cw], BF16, tag="tsb")
                    nc.vector.tensor_sub(tsb[:], st["o2sb"][:], mups[:])
                    varsb = ret.tile([128, cw], F32R, tag="varsb")
                    nc.vector.tensor_sub(varsb[:], msps[:], mu2[:])
                    rstd = ret.tile([128, cw], BF16, tag="rstd")
                    # 1/sqrt(var+eps) in one table-resident activation; the
                    # abs is a no-op since var+eps > 0
                    nc.scalar.activation(rstd[:], varsb[:],
                                         AF.Abs_reciprocal_sqrt, bias=col(4 * L))
                    blip(tsb)
                    # hr = (o2-mu)*rstd*gn_g + gn_b; the gn_b term is folded
                    # into the w2 bias on the host, so one stt does the rest
                    st["hrT"] = ret.tile([128, cw], BF16, name="hrT", tag="hrT")
                    nc.vector.scalar_tensor_tensor(
                        st["hrT"][:], tsb[:], col(4 * l + 2), rstd[:],
                        mybir.AluOpType.mult, mybir.AluOpType.mult,
                    )

                def seg4():
                    h2ps = pp.tile([128, cw], F32, tag="ps")
                    nc.tensor.matmul(h2ps[:], w2a3[:, l, :], st["hrT"][:],
                                     start=True, stop=True)
                    nc.scalar.activation(
                        dest, h2ps[:], AF.Relu,
                        bias=col(4 * l + 1), scale=1.0,
                    )

                return seg1, seg2, seg3, seg4

            def transpose_tiles(srcsb, pairs):
                """PE-transpose [c, node] 128-blocks into node-major hnat
                tiles. pairs = [(src_block_idx, hnat_tile_idx), ...]."""
                for sb_i, mt in pairs:
                    # fp8 PE transpose requires output element step 2
                    tp = pp.tile([128, 256], F8, tag="ps")
                    nc.tensor.transpose(
                        tp[:, 0:256:2], srcsb[:, sb_i * 128 : (sb_i + 1) * 128],
                        ident8sb[:],
                    )
                    copy_alt(hnat[:, mt * 128 : (mt + 1) * 128], tp[:, 0:256:2])

            # ---------- layers ----------
            for l in range(L):
                (a0, aw), (b0, bw) = CHUNKS
                if l > 0:
                    # the A-half of the gather (2 of 3 tiles per shard) lands
                    # first: rebuild+diffuse those while AG-B is in flight
                    hshA3 = hshA.rearrange("p (s j) -> p s j", s=4)
                    hshB3 = hshB.rearrange("p (s j) -> p s j", s=4)
                    nc.sync.dma_start(
                        hshA3[:, 0:2, :],
                        gA_out[l - 1][0:256, :].rearrange("(s c) j -> c s j", s=2),
                    )
                    nc.scalar.dma_start(
                        hshA3[:, 2:4, :],
                        gA_out[l - 1][256:512, :].rearrange("(s c) j -> c s j", s=2),
                    )
                    nc.gpsimd.dma_start(
                        hshB3[:, 0:2, :],
                        gB_out[l - 1][0:256, :].rearrange("(s c) j -> c s j", s=2),
                    )
                    nc.sync.dma_start(
                        hshB3[:, 2:4, :],
                        gB_out[l - 1][256:512, :].rearrange("(s c) j -> c s j", s=2),
                    )
                    transpose_tiles(
                        hshA, [(2 * s + t, 3 * s + t) for s in range(4)
                               for t in (0, 1)]
                    )

                if l == 2:
                    hnT_full = big.tile([C, NSP], F8, tag="hnT")
                    destA = hnT_full[:, a0 : a0 + aw]
                    destB = hnT_full[:, b0 : b0 + bw]
                else:
                    hnTa = big.tile([C, 256], F8, tag="hnTa")
                    hnTb = big.tile([C, 128], F8, tag="hnTb")
                    destA = hnTa[:, :]
                    destB = hnTb[:, 0:bw]

                mpsA = pm.tile([128, aw], F32, tag="mps")
                if l == 0:
                    zA1 = diff_mms(RGROUPS[0], a0, aw)
                else:
                    # start on the early tiles; transpose the B-half tiles as
                    # soon as AG-B delivers, then finish the accumulation
                    zA1 = diff_mms(RGROUPS[0], a0, aw, mts=EARLY, last=False)
                    transpose_tiles(hshB, [(s, 3 * s + 2) for s in range(4)])
                    diff_mms(RGROUPS[0], a0, aw, mts=LATE, zps=zA1, first=False)
                diff_proj(l, RGROUPS[0], zA1, mpsA, aw)
                zA2 = diff_mms(RGROUPS[1], a0, aw)
                diff_proj(l, RGROUPS[1], zA2, mpsA, aw)
                qA = ret_head(l, mpsA, a0, aw)
                # chunk B diffusion interleaved with chunk A retention tail:
                # A's DVE/ACT chain runs while the PE grinds B's adjacency
                # matmuls; A's few PE hops slot between B's r-groups.
                s1, s2, s3, s4 = ret_tail(l, *qA, a0, aw, destA)
                mpsB = pm.tile([128, bw], F32, tag="mps")
                zB1 = diff_mms(RGROUPS[0], b0, bw)
                s1()
                diff_proj(l, RGROUPS[0], zB1, mpsB, bw)
                s2()
                zB2 = diff_mms(RGROUPS[1], b0, bw)
                s3()
                diff_proj(l, RGROUPS[1], zB2, mpsB, bw)
                s4()
                if l < 2:
                    # ship the A half while chunk B's retention chain runs
                    nc.sync.dma_start(gA_in[l][:, :], hnTa[:])
                    nc.gpsimd.collective_compute(
                        "AllGather", mybir.AluOpType.bypass,
                        replica_groups=RG,
                        ins=[gA_in[l][:, :].opt()],
                        outs=[gA_out[l][:, :].opt()],
                    )
                qB = ret_head(l, mpsB, b0, bw)
                t1, t2, t3, t4 = ret_tail(l, *qB, b0, bw, destB)
                t1(); t2(); t3(); t4()

                if l < 2:
                    nc.scalar.dma_start(gB_in[l][:, 0:bw], hnTb[:, 0:bw])
                    nc.gpsimd.collective_compute(
                        "AllGather", mybir.AluOpType.bypass,
                        replica_groups=RG,
                        ins=[gB_in[l][:, :].opt()],
                        outs=[gB_out[l][:, :].opt()],
                    )
                else:
                    # final head
                    hmps = pp.tile([128, NSP], F32, tag="ps")
                    nc.tensor.matmul(hmps[:], ow1tsb[:], hnT_full[:],
                                     start=True, stop=True)
                    hmsb = misc.tile([C, NSP], BF16, tag="hmsb")
                    nc.scalar.activation(
                        hmsb[:], hmps[:], AF.Relu, bias=col(4 * L + 1)
                    )
                    oops = pp.tile([OUT, NSP], F32, tag="ps")
                    nc.tensor.matmul(oops[:], ow2tsb[:], hmsb[:],
                                     start=True, stop=True)
                    oosb = misc.tile([OUT, NSP], F32R, tag="oosb")
                    nc.scalar.activation(
                        oosb[:], oops[:], AF.Identity,
                        bias=colsb[0:OUT, 4 * L + 2 : 4 * L + 3],
                    )
                    nc.sync.dma_start(outt[:, :], oosb[:])

    nc.finalize()
    _NC_CACHE["nc"] = nc
    return nc


def _prep(inputs):
    import ml_dtypes

    bf16 = ml_dtypes.bfloat16
    f8 = ml_dtypes.float8_e4m3
    f32 = np.float32

    def g(name):
        return np.asarray(inputs[name], f32)

    x, adj = g("x"), g("adj_list")
    alpha, transition = g("alpha"), g("transition")
    conv_w, conv_b = g("conv_w"), g("conv_b")
    w1, b1, eb1 = g("w1"), g("b1"), g("eb1")
    w2, b2, eb2 = g("w2"), g("b2"), g("eb2")

    a = alpha - alpha.max(-1, keepdims=True)
    e = np.exp(a)
    srow = (e / e.sum(-1, keepdims=True)).sum(-1)          # [L,R]
    Wm = transition.mean(axis=2)                            # [L,R,C,C]
    Wp = (conv_w * srow)[:, :, None, None] * np.swapaxes(Wm, -1, -2)

    hp = np.zeros((C,), f32)
    b2eff = np.zeros((L, C), f32)
    for l in range(L):
        # gn_b's contribution through w2 is folded in here so the kernel's
        # GN affine is a single (x-mu)*rstd*gn_g op
        b2eff[l] = b2[l] + eb2[l] + w2[l][:, C:] @ hp + w2[l][:, :C] @ g("gn_b")[l]
        hp = np.maximum(hp @ w1[l].T + b1[l] + eb1[l], 0.0).astype(f32)

    qkvo = np.stack(
        [np.swapaxes(g(w), -1, -2) for w in ("qw", "kw", "vw", "ow")], axis=1
    )  # [L,4,C,C] in lhsT layout

    hid = np.arange(C) // HD
    same = (hid[:, None] == hid[None, :]).astype(f32)       # [C,C]

    cols = np.zeros((C, 4 * L + 3), f32)
    cols2 = np.zeros((C, 4 * L), f32)
    rows = np.zeros((1, 4 * L * C), f32)
    for l in range(L):
        cols[:, 4 * l + 0] = conv_b[l]
        cols[:, 4 * l + 1] = b2eff[l]
        cols[:, 4 * l + 2] = g("gn_g")[l]
        cols[:, 4 * l + 3] = g("gn_b")[l]
        for j, nm in enumerate(("qb", "kb", "vb", "ob")):
            cols2[:, 4 * l + j] = g(nm)[l]
            rows[0, (4 * l + j) * C : (4 * l + j + 1) * C] = g(nm)[l]
    cols[:, 4 * L] = EPS
    cols[:, 4 * L + 1] = g("out_b1")
    cols[:OUT, 4 * L + 2] = g("out_b2")

    wp_h = np.ascontiguousarray(
        (Wp / 16.0).transpose(2, 0, 1, 3).reshape(C, L * R * C)
    )
    qkvo_h = np.ascontiguousarray(qkvo.transpose(2, 0, 1, 3).reshape(C, L * 4 * C))
    w2at_h = np.ascontiguousarray(
        np.swapaxes(w2[:, :, :C], -1, -2).transpose(1, 0, 2).reshape(C, L * C)
    )
    eye = np.eye(C, dtype=f32)
    consts = {
        "cb": np.concatenate(
            [wp_h, qkvo_h, same, w2at_h, eye, g("out_w2").T], axis=1
        ).astype(bf16),
        "cf": np.concatenate([cols, cols2], axis=1).astype(f32),
        "mmu": (same / HD).astype(f32),
        "c8": np.concatenate([eye, g("out_w1").T], axis=1).astype(f8),
        "embt": np.concatenate(
            [g("emb_w").T, g("emb_b")[None, :]], axis=0
        ).astype(bf16),
        "rows": rows.astype(bf16),
    }

    xlast = x[:, :, -1, :]                                   # [B,N,DIN]
    in_maps = []
    for k in range(NCORES):
        b, s = k // 4, k % 4
        asub = adj[b][:, s * NS : (s + 1) * NS, :]           # [R,NS,N] dest rows
        ap = np.zeros((R, NSP, MPAD), f32)
        for s2 in range(4):
            ap[:, :NS, s2 * NSH : s2 * NSH + NS] = asub[:, :, s2 * NS : (s2 + 1) * NS]
        a3 = ap.transpose(2, 0, 1).reshape(NT, 128, R, NSP)  # [mt, mi, R, NSP]
        a3 = (
            a3.reshape(2, NT // 2, 128, R * NSP)
            .transpose(0, 2, 1, 3)                           # [half, mi, 6, R*NSP]
            .reshape(2, 128, (NT // 2) * R * NSP)
        )
        xt = np.zeros((DIN + 1, MPAD), f32)
        for s2 in range(4):
            xt[:DIN, s2 * NSH : s2 * NSH + NS] = xlast[b, s2 * NS : (s2 + 1) * NS].T
        xt[DIN, :] = 1.0
        in_maps.append(
            dict(consts, adjt=(np.ascontiguousarray(a3) * 16.0).astype(f8),
                 xt=xt.astype(bf16))
        )
    return in_maps


def kernel(**inputs):
    nc = _build_nc()
    in_maps = _prep(inputs)
    res = run_bass_kernel_spmd(nc, in_maps, core_ids=list(range(NCORES)))
    out = np.zeros((B, N, OUT), np.float32)
    for k in range(NCORES):
        b, s = k // 4, k % 4
        out[b, s * NS : (s + 1) * NS, :] = res.results[k]["outt"][:, :NS].T
    return out
